# revision 1
# baseline (speedup 1.0000x reference)
"""Trainium2 Bass kernel for nn_AttentionPool (segment softmax-pool over gene/spot edges).

Math: out[g] = (sum_{s in S_g} e_s * emb[s]) / (sum_{s in S_g} e_s),
      e_s = exp(logit_s - 30),  logit = tanh(emb @ W.T + b) @ v
where S_g is the *set* of distinct spots expressing gene g (duplicate edges
count once), and empty genes produce 0. The row-max shift of the reference
softmax cancels; the constant -30 shift keeps exp() in fp32 range
(|logit| <= sum|v| < 27.6 for this problem's xavier init).

Sharding: 2500 genes per core x 8 cores (padded to 2560 = 20 tiles of 128).
Host marshals the edge list into each core's dense 0/1 mask slab, laid out
as [20 gene-tiles, 128 spot-partition, 32 spot-chunk, 128 gene] bf16 so each
strip is one contiguous 1MB DMA and each [128,128] chunk is a matmul lhsT.
All floating-point math runs on device. X is carried as bf16 hi+lo pairs so
the bf16 matmuls reproduce fp32 accuracy (~1e-6).
"""

import sys

sys.path.insert(0, "/opt/trn_rl_repo")

import numpy as np
import ml_dtypes

import concourse.mybir as mybir
import concourse.tile as tile
from concourse import bacc
from concourse.bass import ts
from concourse.tile import add_dep_helper
from concourse.bass_utils import run_bass_kernel_spmd
from concourse.bass_interp import get_hw_module

F32 = mybir.dt.float32
BF16 = mybir.dt.bfloat16
U8 = mybir.dt.uint8

N_SPOTS = 4096
N_GENES = 20000
D = 128
N_CORES = 8
G_PER = N_GENES // N_CORES  # 2500
P = 128
KCH = N_SPOTS // P  # 32 spot chunks
NX = 258  # [Xhi | Xlo] columns: 2 * (D + 1)


def build_nc(T, wide_mm=False):
    """Build the single-core Bass program (SPMD across 8 cores).

    T = number of 128-gene tiles per core (20 for the real problem).
    wide_mm = one N=258 matmul per chunk (single LDWEIGHTS) instead of two
    N=129 matmuls sharing PSUM columns.
    """
    nc = bacc.Bacc("TRN2", target_bir_lowering=False, debug=False, num_devices=N_CORES)

    maskbt = nc.dram_tensor("maskbt", [T, P, KCH * P], U8, kind="ExternalInput")
    # emb pre-swizzled on host to spot-partition layout: [p, k*128+d] =
    # emb[k*128+p, d] -> each SBUF partition line is one contiguous 16KB read
    embcp = nc.dram_tensor("embcp", [P, KCH * D], F32, kind="ExternalInput")
    embT = nc.dram_tensor("embT", [D, N_SPOTS], F32, kind="ExternalInput")
    wt = nc.dram_tensor("wt", [D, D], F32, kind="ExternalInput")
    bb = nc.dram_tensor("bb", [D, 1], F32, kind="ExternalInput")
    vv = nc.dram_tensor("vv", [D, 1], F32, kind="ExternalInput")
    out = nc.dram_tensor("out", [T, P, D], F32, kind="ExternalOutput")

    with tile.TileContext(nc) as tc:
        with (
            tc.tile_pool(name="const", bufs=1) as constp,
            tc.tile_pool(name="xfp", bufs=1) as xfp,
            tc.tile_pool(name="maskp", bufs=5) as maskp,
            tc.tile_pool(name="outp", bufs=2) as outp,
            tc.tile_pool(name="php", bufs=4, space="PSUM") as php,
            tc.tile_pool(name="pep", bufs=1, space="PSUM") as pep,
            tc.tile_pool(name="ptp", bufs=3, space="PSUM") as ptp,
        ):
            # ---- constants into SBUF ----
            wt_sb = constp.tile([P, D], F32)
            nc.sync.dma_start(out=wt_sb[:], in_=wt[:])
            b_sb = constp.tile([P, 1], F32)
            nc.sync.dma_start(out=b_sb[:], in_=bb[:])
            v_sb = constp.tile([P, 1], F32)
            nc.sync.dma_start(out=v_sb[:], in_=vv[:])
            # big loads split in halves across both HWDGE rings, each half its
            # own tile so downstream compute starts as soon as its half lands
            HS = N_SPOTS // 2
            embT_a = constp.tile([P, HS], F32)
            embT_b = constp.tile([P, HS], F32)
            embT_dma1 = nc.sync.dma_start(out=embT_a[:], in_=embT[:, 0:HS])
            embT_dma2 = nc.scalar.dma_start(out=embT_b[:], in_=embT[:, HS:])

            def embT_cols(lo, width):
                # view into the correct half-tile (never straddles: callers
                # use 512- or 128-aligned slices within one half)
                if lo < HS:
                    return embT_a[:, lo : lo + width]
                return embT_b[:, lo - HS : lo - HS + width]

            neg30 = constp.tile([P, 1], F32)
            nc.gpsimd.memset(neg30[:], -30.0)

            th_sb = constp.tile([P, N_SPOTS], F32)  # tanh(W h + b).T  [j, s]
            e_sb = constp.tile([P, KCH], F32)  # e in spot-partition layout
            xhl = constp.tile([P, KCH * NX], BF16)  # [Xhi | Xlo] per chunk

            # ---- prologue: logits ----
            # h.T [j, s] = (W.T).T @ emb.T ; tanh(+b) fused from PSUM
            for c in range(N_SPOTS // 512):
                ph = php.tile([P, 512], F32)
                nc.tensor.matmul(
                    out=ph[:], lhsT=wt_sb[:], rhs=embT_cols(c * 512, 512),
                    start=True, stop=True,
                )
                nc.scalar.activation(
                    out=th_sb[:, ts(c, 512)], in_=ph[:],
                    func=mybir.ActivationFunctionType.Tanh, bias=b_sb[:, 0:1],
                )
            # logits, transposed into spot-partition layout:
            # logitsT chunk [128 s, 1] = th_chunk[j, s].T @ v
            pe = pep.tile([P, KCH], F32)
            for k in range(KCH):
                nc.tensor.matmul(
                    out=pe[:, k : k + 1], lhsT=th_sb[:, ts(k, P)], rhs=v_sb[:],
                    start=True, stop=True,
                )
            nc.scalar.activation(
                out=e_sb[:], in_=pe[:],
                func=mybir.ActivationFunctionType.Exp, bias=neg30[:, 0:1],
            )

            # ---- X = [e*emb | e] as bf16 hi + lo (full-width batched ops) ----
            # emb in spot-partition layout; scalar HWDGE ring so it runs in
            # parallel with the embT load on the sync ring
            HC = KCH * D // 2
            embc_a = constp.tile([P, HC], F32)
            embc_b = constp.tile([P, HC], F32)
            embc_dma1 = nc.sync.dma_start(out=embc_a[:], in_=embcp[:, 0:HC])
            embc_dma2 = nc.scalar.dma_start(out=embc_b[:], in_=embcp[:, HC:])
            xf = xfp.tile([P, KCH * D], F32)
            xhl3 = xhl[:].rearrange("p (k n) -> p k n", n=NX)
            xf3 = xf[:].rearrange("p (k d) -> p k d", d=D)
            emb3a = embc_a[:].rearrange("p (k d) -> p k d", d=D)
            emb3b = embc_b[:].rearrange("p (k d) -> p k d", d=D)
            e3 = e_sb[:].rearrange("p k -> p k ()")
            NG = 4  # build X in chunk groups so the main loop starts early
            GS = KCH // NG
            for g in range(NG):
                ks = slice(g * GS, (g + 1) * GS)
                if g < NG // 2:
                    embsrc = emb3a[:, ks, :]
                else:
                    embsrc = emb3b[:, slice(g * GS - KCH // 2, (g + 1) * GS - KCH // 2), :]
                ebc = e3[:, ks, :].to_broadcast([P, GS, D])
                nc.vector.tensor_mul(out=xf3[:, ks, :], in0=embsrc, in1=ebc)
                hi3 = xhl3[:, ks, 0:D]
                nc.scalar.activation(
                    out=hi3, in_=xf3[:, ks, :], func=mybir.ActivationFunctionType.Copy
                )
                nc.vector.tensor_sub(
                    out=xhl3[:, ks, D + 1 : NX - 1], in0=xf3[:, ks, :], in1=hi3
                )
                nc.vector.tensor_copy(out=xhl3[:, ks, D : D + 1], in_=e3[:, ks, :])
                nc.vector.tensor_sub(
                    out=xhl3[:, ks, NX - 1 : NX], in0=e3[:, ks, :],
                    in1=xhl3[:, ks, D : D + 1],
                )

            # ---- main loop: per gene tile ----
            for t in range(T):
                mt = maskp.tile([P, KCH * P], BF16, name=f"mt{t}", tag="mt")
                mdma = nc.gpsimd.dma_start(out=mt[:], in_=maskbt[t])  # u8->bf16 cast
                if t < 4:
                    # keep the prefetch burst from stealing SDMA engines
                    # while the latency-critical emb loads are in flight
                    for dep in (embT_dma1, embT_dma2, embc_dma1, embc_dma2):
                        add_dep_helper(mdma.ins, dep.ins, True, "mask after emb")
                if wide_mm:
                    # one LDW + one N=258 matmul per chunk; hi/lo halves
                    # summed on DVE afterwards
                    pt = ptp.tile([P, NX], F32, name=f"ptw{t}", tag="pt")
                    for k in range(KCH):
                        nc.tensor.matmul(
                            out=pt[:], lhsT=mt[:, ts(k, P)], rhs=xhl[:, ts(k, NX)],
                            start=(k == 0), stop=(k == KCH - 1),
                        )
                    lo_sb = outp.tile([P, D + 1], F32, tag="lo_sb")
                    nc.vector.tensor_copy(out=lo_sb[:], in_=pt[:, D + 1 : NX])
                    s_sb = outp.tile([P, D + 1], F32, tag="s_sb")
                    nc.vector.tensor_add(out=s_sb[:], in0=pt[:, 0 : D + 1], in1=lo_sb[:])
                else:
                    # hi and lo accumulate into the same PSUM columns
                    pt = ptp.tile([P, D + 1], F32, name=f"ptn{t}", tag="pt")
                    for k in range(KCH):
                        nc.tensor.matmul(
                            out=pt[:], lhsT=mt[:, ts(k, P)],
                            rhs=xhl[:, k * NX : k * NX + (D + 1)],
                            start=(k == 0), stop=False,
                        )
                        nc.tensor.matmul(
                            out=pt[:], lhsT=mt[:, ts(k, P)],
                            rhs=xhl[:, k * NX + (D + 1) : (k + 1) * NX],
                            start=False, stop=(k == KCH - 1),
                        )
                    s_sb = pt
                rmax = outp.tile([P, 1], F32, tag="rmax")
                nc.vector.tensor_scalar_max(out=rmax[:], in0=s_sb[:, D : D + 1], scalar1=1e-37)
                rinv = outp.tile([P, 1], F32, tag="rinv")
                nc.vector.reciprocal(out=rinv[:], in_=rmax[:])
                o = outp.tile([P, D], F32, tag="o")
                nc.vector.tensor_scalar_mul(out=o[:], in0=s_sb[:, 0:D], scalar1=rinv[:, 0:1])
                nc.sync.dma_start(out=out[t], in_=o[:])

    nc.compile()
    return nc


def prep_inputs(spot_emb, W, b, v, gene_ids, spot_ids, T):
    """Host marshaling: shared fp32 operands + per-core mask slabs."""
    emb = np.ascontiguousarray(np.asarray(spot_emb, dtype=np.float32))
    W = np.asarray(W, dtype=np.float32)
    b = np.asarray(b, dtype=np.float32)
    v = np.asarray(v, dtype=np.float32)
    gene_ids = np.asarray(gene_ids).astype(np.int64)
    spot_ids = np.asarray(spot_ids).astype(np.int64)

    shared = {
        "embcp": np.ascontiguousarray(
            emb.reshape(KCH, P, D).transpose(1, 0, 2).reshape(P, KCH * D)
        ),
        "embT": np.ascontiguousarray(emb.T),
        "wt": np.ascontiguousarray(W.T),
        "bb": np.ascontiguousarray(b.reshape(D, 1)),
        "vv": np.ascontiguousarray(v.reshape(D, 1)),
    }

    # Dense 0/1 occupancy mask (set semantics: duplicate edges collapse),
    # built directly in the per-core padded layout: core c's genes live at
    # rows [c*T*P, c*T*P + G_PER); rows above G_PER stay zero padding.
    g_pad = T * P
    M = np.zeros((N_CORES * g_pad, N_SPOTS), dtype=bool)
    pad_rows = (gene_ids // G_PER) * g_pad + (gene_ids % G_PER)
    M[pad_rows, spot_ids] = True
    # [c, t*128+g, k*128+p] -> [c, t, p, k, g]
    Mbt = M.reshape(N_CORES, T, P, KCH, P).transpose(0, 1, 4, 3, 2)
    Mbt = np.ascontiguousarray(Mbt).astype(np.uint8).reshape(N_CORES, T, P, KCH * P)
    return [{"maskbt": Mbt[c], **shared} for c in range(N_CORES)]


_NC_CACHE = {}


def run(spot_emb, W, b, v, gene_ids, spot_ids, trace=False, wide_mm=False, **hw_kwargs):
    T = (G_PER + P - 1) // P  # 20
    key = (T, wide_mm)
    if key not in _NC_CACHE:
        nc = build_nc(T, wide_mm=wide_mm)
        nc.m = get_hw_module(nc.m)
        _NC_CACHE[key] = nc
    nc = _NC_CACHE[key]
    in_maps = prep_inputs(spot_emb, W, b, v, gene_ids, spot_ids, T)
    res = run_bass_kernel_spmd(
        nc, in_maps, core_ids=list(range(N_CORES)), trace=trace, **hw_kwargs
    )
    outs = [
        np.asarray(res.results[c]["out"], dtype=np.float32).reshape(T * P, D)[:G_PER]
        for c in range(N_CORES)
    ]
    full = np.concatenate(outs, axis=0)
    return full, res


def kernel(spot_emb, W, b, v, gene_ids, spot_ids, n_genes):
    n_genes = int(n_genes)
    assert n_genes == N_GENES, f"kernel hardcodes n_genes={N_GENES}, got {n_genes}"
    full, _ = run(spot_emb, W, b, v, gene_ids, spot_ids, trace=False)
    return full



# revision 5
# speedup vs baseline: 1.0671x; 1.0671x over previous
"""Trainium2 Bass kernel for nn_AttentionPool (segment softmax-pool over gene/spot edges).

Math: out[g] = (sum_{s in S_g} e_s * emb[s]) / (sum_{s in S_g} e_s),
      e_s = exp(logit_s),  logit = tanh(emb @ W.T + b) @ v
where S_g is the *set* of distinct spots expressing gene g (duplicate edges
count once), and empty genes produce 0.

The softmax shift cancels in the num/den ratio, so e is rescaled on device by
s = 256 / sum(e) to center values in fp8 range: X = [s*e*emb | s*e] is stored
as fp8e4m3 hi+lo pairs (~7 mantissa bits combined), and the 0/1 gene-spot
mask is fp8-exact. The pooling matmul then runs in DoubleRow perf mode
(2 spot-chunks contracted per instruction at 0.5 cyc/row — 4x bf16), and the
mask DMA moves raw fp8 bytes (half the SBUF write traffic of the old
u8->bf16 cast path, and no cast).

Sharding: 2500 genes per core x 8 cores (padded to 2560 = 20 tiles of 128).
Host marshals the edge list into each core's dense 0/1 fp8 mask slab, laid
out [20 gene-tiles, 128 spot-partition, 32 spot-chunk, 128 gene] so each
[128, 2, 128] slice is a DoubleRow lhsT. Mask tiles round-robin across the
three DMA queues (gpsimd SW-DGE + SP/ACT HW-DGE).
"""

import sys

sys.path.insert(0, "/opt/trn_rl_repo")

import numpy as np
import ml_dtypes

import concourse.mybir as mybir
import concourse.tile as tile
from concourse import bacc
from concourse.bass import ts
from concourse.bass_utils import run_bass_kernel_spmd
from concourse.bass_interp import get_hw_module

F32 = mybir.dt.float32
F8 = mybir.dt.float8e4
DR = mybir.MatmulPerfMode.DoubleRow

N_SPOTS = 4096
N_GENES = 20000
D = 128
N_CORES = 8
G_PER = N_GENES // N_CORES  # 2500
P = 128
KCH = N_SPOTS // P  # 32 spot chunks
NX = 258  # per-chunk X columns: [hi(129) | lo(129)]
ESCALE = 256.0  # e rescale numerator: es = 256*e/sum(e)


def build_nc(T):
    """Build the single-core Bass program (SPMD across 8 cores).

    T = number of 128-gene tiles per core (20 for the real problem).
    """
    nc = bacc.Bacc("TRN2", target_bir_lowering=False, debug=False, num_devices=N_CORES)

    maskbt = nc.dram_tensor("maskbt", [T, P, KCH * P], F8, kind="ExternalInput")
    # emb pre-swizzled on host to spot-partition layout: [p, k*128+d] =
    # emb[k*128+p, d] -> each SBUF partition line is one contiguous 16KB read
    embcp = nc.dram_tensor("embcp", [P, KCH * D], F32, kind="ExternalInput")
    embT = nc.dram_tensor("embT", [D, N_SPOTS], F32, kind="ExternalInput")
    wt = nc.dram_tensor("wt", [D, D], F32, kind="ExternalInput")
    bb = nc.dram_tensor("bb", [D, 1], F32, kind="ExternalInput")
    vv = nc.dram_tensor("vv", [D, 1], F32, kind="ExternalInput")
    out = nc.dram_tensor("out", [T, P, D], F32, kind="ExternalOutput")

    with tile.TileContext(nc) as tc:
        with (
            tc.tile_pool(name="const", bufs=1) as constp,
            tc.tile_pool(name="xfp", bufs=1) as xfp,
            tc.tile_pool(name="maskp", bufs=8) as maskp,
            tc.tile_pool(name="outp", bufs=2) as outp,
            tc.tile_pool(name="php", bufs=2, space="PSUM") as php,
            tc.tile_pool(name="pep", bufs=1, space="PSUM") as pep,
            tc.tile_pool(name="psp", bufs=1, space="PSUM") as psp,
            tc.tile_pool(name="ptp", bufs=3, space="PSUM") as ptp,
        ):
            # ---- constants into SBUF ----
            wt_sb = constp.tile([P, D], F32)
            nc.sync.dma_start(out=wt_sb[:], in_=wt[:])
            b_sb = constp.tile([P, 1], F32)
            nc.sync.dma_start(out=b_sb[:], in_=bb[:])
            v_sb = constp.tile([P, 1], F32)
            nc.sync.dma_start(out=v_sb[:], in_=vv[:])
            ones_c = constp.tile([P, 1], F32)  # lhsT for total-sum matmul
            nc.gpsimd.memset(ones_c[:], 1.0)
            ones_r = constp.tile([1, P], F32)  # lhsT for scalar broadcast
            nc.gpsimd.memset(ones_r[:], 1.0)

            # big loads split in halves across both HWDGE rings, each half its
            # own tile so downstream compute starts as soon as its half lands
            HS = N_SPOTS // 2
            embT_a = constp.tile([P, HS], F32)
            embT_b = constp.tile([P, HS], F32)
            nc.sync.dma_start(out=embT_a[:], in_=embT[:, 0:HS])
            nc.scalar.dma_start(out=embT_b[:], in_=embT[:, HS:])

            def embT_cols(lo, width):
                # view into the correct half-tile (never straddles: callers
                # use 512-aligned slices within one half)
                if lo < HS:
                    return embT_a[:, lo : lo + width]
                return embT_b[:, lo - HS : lo - HS + width]

            th_sb = constp.tile([P, N_SPOTS], F32)  # tanh(W h + b).T  [j, s]
            e_sb = constp.tile([P, KCH], F32)  # e in spot-partition layout
            es_sb = constp.tile([P, KCH], F32)  # rescaled e
            rowsum = constp.tile([P, 1], F32)
            tot_sb = constp.tile([1, 1], F32)
            sinv_sb = constp.tile([1, 1], F32)
            ss_sb = constp.tile([P, 1], F32)  # broadcast scale
            xhl = constp.tile([P, KCH * NX], F8)  # [Xhi es_hi | Xlo es_lo]/chunk

            # ---- prologue: logits ----
            # h.T [j, s] = (W.T).T @ emb.T ; tanh(+b) fused from PSUM
            for c in range(N_SPOTS // 512):
                ph = php.tile([P, 512], F32)
                nc.tensor.matmul(
                    out=ph[:], lhsT=wt_sb[:], rhs=embT_cols(c * 512, 512),
                    start=True, stop=True,
                )
                nc.scalar.activation(
                    out=th_sb[:, ts(c, 512)], in_=ph[:],
                    func=mybir.ActivationFunctionType.Tanh, bias=b_sb[:, 0:1],
                )
            # logits, transposed into spot-partition layout:
            # logitsT chunk [128 s, 1] = th_chunk[j, s].T @ v
            pe = pep.tile([P, KCH], F32)
            for k in range(KCH):
                nc.tensor.matmul(
                    out=pe[:, k : k + 1], lhsT=th_sb[:, ts(k, P)], rhs=v_sb[:],
                    start=True, stop=True,
                )
            # e = exp(logits); accum_out gives the per-partition row sum free
            nc.scalar.activation(
                out=e_sb[:], in_=pe[:],
                func=mybir.ActivationFunctionType.Exp, accum_out=rowsum[:],
            )
            # global scale s = ESCALE / sum(e): partition-sum via ones matmul,
            # then reciprocal, then broadcast back across partitions
            ptot = psp.tile([1, 1], F32)
            nc.tensor.matmul(
                out=ptot[:], lhsT=ones_c[:, 0:1], rhs=rowsum[:, 0:1],
                start=True, stop=True,
            )
            nc.vector.tensor_copy(out=tot_sb[:], in_=ptot[:])
            nc.vector.reciprocal(out=sinv_sb[:], in_=tot_sb[:])
            pbc = psp.tile([P, 1], F32)
            nc.tensor.matmul(
                out=pbc[:], lhsT=ones_r[0:1, :], rhs=sinv_sb[0:1, 0:1],
                start=True, stop=True,
            )
            nc.vector.tensor_scalar_mul(out=ss_sb[:], in0=pbc[:], scalar1=ESCALE)
            nc.vector.tensor_scalar_mul(
                out=es_sb[:], in0=e_sb[:], scalar1=ss_sb[:, 0:1]
            )

            # ---- X = [es*emb | es] as fp8 hi + lo (batched over chunk groups) ----
            # emb in spot-partition layout; scalar HWDGE ring so it runs in
            # parallel with the embT load on the sync ring
            HC = KCH * D // 2
            embc_a = constp.tile([P, HC], F32)
            embc_b = constp.tile([P, HC], F32)
            nc.sync.dma_start(out=embc_a[:], in_=embcp[:, 0:HC])
            nc.scalar.dma_start(out=embc_b[:], in_=embcp[:, HC:])
            xf = xfp.tile([P, KCH * D], F32)
            xhl3 = xhl[:].rearrange("p (k n) -> p k n", n=NX)
            xf3 = xf[:].rearrange("p (k d) -> p k d", d=D)
            emb3a = embc_a[:].rearrange("p (k d) -> p k d", d=D)
            emb3b = embc_b[:].rearrange("p (k d) -> p k d", d=D)
            es3 = es_sb[:].rearrange("p k -> p k ()")
            NG = 4  # build X in chunk groups so the main loop starts early
            GS = KCH // NG
            for g in range(NG):
                ks = slice(g * GS, (g + 1) * GS)
                if g < NG // 2:
                    embsrc = emb3a[:, ks, :]
                else:
                    embsrc = emb3b[:, slice(g * GS - KCH // 2, (g + 1) * GS - KCH // 2), :]
                ebc = es3[:, ks, :].to_broadcast([P, GS, D])
                nc.vector.tensor_mul(out=xf3[:, ks, :], in0=embsrc, in1=ebc)
                hi3 = xhl3[:, ks, 0:D]
                nc.scalar.activation(
                    out=hi3, in_=xf3[:, ks, :], func=mybir.ActivationFunctionType.Copy
                )
                nc.vector.tensor_sub(
                    out=xhl3[:, ks, D + 1 : NX - 1], in0=xf3[:, ks, :], in1=hi3
                )
                nc.vector.tensor_copy(out=xhl3[:, ks, D : D + 1], in_=es3[:, ks, :])
                nc.vector.tensor_sub(
                    out=xhl3[:, ks, NX - 1 : NX], in0=es3[:, ks, :],
                    in1=xhl3[:, ks, D : D + 1],
                )

            # ---- main loop: per gene tile, DoubleRow fp8 matmuls ----
            # Mask tiles ride the gpsimd SW-DGE and ACT HW-DGE queues only;
            # the sync queue carries the per-tile output DMAs, which would
            # otherwise head-of-line-block later mask tiles (DGE queues are
            # FIFO and out[t] waits on tile t's full compute).
            for t in range(T):
                mt = maskp.tile([P, KCH * P], F8, name=f"mt{t}", tag="mt")
                eng = nc.gpsimd if t % 5 in (0, 2, 4) else nc.scalar
                eng.dma_start(out=mt[:], in_=maskbt[t])
                mt3 = mt[:].rearrange("p (k g) -> p k g", g=P)
                pt = ptp.tile([P, D + 1], F32, name=f"pt{t}", tag="pt")
                for kk in range(0, KCH, 2):
                    k2 = slice(kk, kk + 2)
                    nc.tensor.matmul(
                        out=pt[:], lhsT=mt3[:, k2, :], rhs=xhl3[:, k2, 0 : D + 1],
                        start=(kk == 0), stop=False, perf_mode=DR,
                    )
                    nc.tensor.matmul(
                        out=pt[:], lhsT=mt3[:, k2, :], rhs=xhl3[:, k2, D + 1 : NX],
                        start=False, stop=(kk == KCH - 2), perf_mode=DR,
                    )
                rmax = outp.tile([P, 1], F32, tag="rmax")
                nc.vector.tensor_scalar_max(out=rmax[:], in0=pt[:, D : D + 1], scalar1=1e-37)
                rinv = outp.tile([P, 1], F32, tag="rinv")
                nc.vector.reciprocal(out=rinv[:], in_=rmax[:])
                o = outp.tile([P, D], F32, tag="o")
                nc.vector.tensor_scalar_mul(out=o[:], in0=pt[:, 0:D], scalar1=rinv[:, 0:1])
                nc.sync.dma_start(out=out[t], in_=o[:])

    nc.compile()
    return nc


def prep_inputs(spot_emb, W, b, v, gene_ids, spot_ids, T):
    """Host marshaling: shared fp32 operands + per-core fp8 mask slabs."""
    emb = np.ascontiguousarray(np.asarray(spot_emb, dtype=np.float32))
    W = np.asarray(W, dtype=np.float32)
    b = np.asarray(b, dtype=np.float32)
    v = np.asarray(v, dtype=np.float32)
    gene_ids = np.asarray(gene_ids).astype(np.int64)
    spot_ids = np.asarray(spot_ids).astype(np.int64)

    shared = {
        "embcp": np.ascontiguousarray(
            emb.reshape(KCH, P, D).transpose(1, 0, 2).reshape(P, KCH * D)
        ),
        "embT": np.ascontiguousarray(emb.T),
        "wt": np.ascontiguousarray(W.T),
        "bb": np.ascontiguousarray(b.reshape(D, 1)),
        "vv": np.ascontiguousarray(v.reshape(D, 1)),
    }

    # Dense 0/1 occupancy mask (set semantics: duplicate edges collapse),
    # built directly in the per-core padded layout: core c's genes live at
    # rows [c*T*P, c*T*P + G_PER); rows above G_PER stay zero padding.
    # Stored as the raw fp8e4m3 byte pattern (1.0 -> 0x38) so the DMA is a
    # plain byte copy with no dtype cast.
    g_pad = T * P
    M = np.zeros((N_CORES * g_pad, N_SPOTS), dtype=bool)
    pad_rows = (gene_ids // G_PER) * g_pad + (gene_ids % G_PER)
    M[pad_rows, spot_ids] = True
    # [c, t*128+g, k*128+p] -> [c, t, p, k, g]
    Mbt = M.reshape(N_CORES, T, P, KCH, P).transpose(0, 1, 4, 3, 2)
    Mbt = (
        np.ascontiguousarray(Mbt)
        .astype(np.uint8)
        .__mul__(np.uint8(0x38))
        .view(ml_dtypes.float8_e4m3)
        .reshape(N_CORES, T, P, KCH * P)
    )
    return [{"maskbt": Mbt[c], **shared} for c in range(N_CORES)]


_NC_CACHE = {}


def run(spot_emb, W, b, v, gene_ids, spot_ids, trace=False, **hw_kwargs):
    T = (G_PER + P - 1) // P  # 20
    key = T
    if key not in _NC_CACHE:
        nc = build_nc(T)
        nc.m = get_hw_module(nc.m)
        _NC_CACHE[key] = nc
    nc = _NC_CACHE[key]
    in_maps = prep_inputs(spot_emb, W, b, v, gene_ids, spot_ids, T)
    res = run_bass_kernel_spmd(
        nc, in_maps, core_ids=list(range(N_CORES)), trace=trace, **hw_kwargs
    )
    outs = [
        np.asarray(res.results[c]["out"], dtype=np.float32).reshape(T * P, D)[:G_PER]
        for c in range(N_CORES)
    ]
    full = np.concatenate(outs, axis=0)
    return full, res


def kernel(spot_emb, W, b, v, gene_ids, spot_ids, n_genes):
    n_genes = int(n_genes)
    assert n_genes == N_GENES, f"kernel hardcodes n_genes={N_GENES}, got {n_genes}"
    full, _ = run(spot_emb, W, b, v, gene_ids, spot_ids, trace=False)
    return full


# revision 11
# speedup vs baseline: 1.3086x; 1.2263x over previous
"""Trainium2 Bass kernel for nn_AttentionPool (segment softmax-pool over gene/spot edges).

Math: out[g] = (sum_{s in S_g} e_s * emb[s]) / (sum_{s in S_g} e_s),
      e_s = exp(logit_s),  logit = tanh(emb @ W.T + b) @ v
where S_g is the *set* of distinct spots expressing gene g (duplicate edges
count once), and empty genes produce 0.

The softmax shift cancels in the num/den ratio, so e is rescaled on device by
s = 256 / sum(e) to center values in fp8 range: X = [s*e*emb | s*e] is stored
as fp8e4m3 hi+lo pairs (~7 mantissa bits combined), and the 0/1 gene-spot
mask is fp8-exact. The pooling matmul then runs in DoubleRow perf mode
(2 spot-chunks contracted per instruction at 0.5 cyc/row — 4x bf16), and the
mask DMA moves raw fp8 bytes (half the SBUF write traffic of the old
u8->bf16 cast path, and no cast).

Sharding: 2500 genes per core x 8 cores (padded to 2560 = 20 tiles of 128).
Host marshals the edge list into each core's dense 0/1 fp8 mask slab, laid
out [20 gene-tiles, 128 spot-partition, 32 spot-chunk, 128 gene] so each
[128, 2, 128] slice is a DoubleRow lhsT. Mask tiles round-robin across the
three DMA queues (gpsimd SW-DGE + SP/ACT HW-DGE).
"""

import sys

sys.path.insert(0, "/opt/trn_rl_repo")

import numpy as np
import ml_dtypes

import concourse.mybir as mybir
import concourse.tile as tile
from concourse import bacc
from concourse.bass import ts
from concourse.bass_utils import run_bass_kernel_spmd
from concourse.bass_interp import get_hw_module

F32 = mybir.dt.float32
F8 = mybir.dt.float8e4
DR = mybir.MatmulPerfMode.DoubleRow
ALU = mybir.AluOpType

N_SPOTS = 4096
N_GENES = 20000
D = 128
N_CORES = 8
G_PER = N_GENES // N_CORES  # 2500
P = 128
KCH = N_SPOTS // P  # 32 spot chunks
NX = 258  # per-chunk X columns: [hi(129) | -64*lo(129)]
ESCALE = 256.0  # e rescale numerator: es = 256*e/sum(e)
LOS = 64.0  # lo residuals stored scaled by 64 so they stay in fp8 normal range


def build_nc(T):
    """Build the single-core Bass program (SPMD across 8 cores).

    T = number of 128-gene tiles per core (20 for the real problem).
    """
    nc = bacc.Bacc("TRN2", target_bir_lowering=False, debug=False, num_devices=N_CORES)

    maskbt = nc.dram_tensor("maskbt", [T, P, KCH * P], F8, kind="ExternalInput")
    # emb pre-swizzled on host to spot-partition layout: [p, k*128+d] =
    # emb[k*128+p, d] -> each SBUF partition line is one contiguous 16KB read
    embcp = nc.dram_tensor("embcp", [P, KCH * D], F32, kind="ExternalInput")
    embT = nc.dram_tensor("embT", [D, N_SPOTS], F32, kind="ExternalInput")
    wt = nc.dram_tensor("wt", [D, D], F32, kind="ExternalInput")
    bb = nc.dram_tensor("bb", [D, 1], F32, kind="ExternalInput")
    vv = nc.dram_tensor("vv", [D, 1], F32, kind="ExternalInput")
    out = nc.dram_tensor("out", [T, P, D], F32, kind="ExternalOutput")

    with tile.TileContext(nc) as tc:
        with (
            tc.tile_pool(name="const", bufs=1) as constp,
            tc.tile_pool(name="xfp", bufs=1) as xfp,
            tc.tile_pool(name="maskp", bufs=20) as maskp,
            tc.tile_pool(name="outp", bufs=2) as outp,
            tc.tile_pool(name="php", bufs=2, space="PSUM") as php,
            tc.tile_pool(name="pep", bufs=1, space="PSUM") as pep,
            tc.tile_pool(name="psp", bufs=1, space="PSUM") as psp,
            tc.tile_pool(name="ptp", bufs=3, space="PSUM") as ptp,
        ):
            # ---- constants into SBUF ----
            wt_sb = constp.tile([P, D], F32)
            nc.sync.dma_start(out=wt_sb[:], in_=wt[:])
            b_sb = constp.tile([P, 1], F32)
            nc.sync.dma_start(out=b_sb[:], in_=bb[:])
            v_sb = constp.tile([P, 1], F32)
            nc.sync.dma_start(out=v_sb[:], in_=vv[:])
            ones_c = constp.tile([P, 1], F32)  # lhsT for total-sum matmul
            nc.gpsimd.memset(ones_c[:], 1.0)
            ones_r = constp.tile([1, P], F32)  # lhsT for scalar broadcast
            nc.gpsimd.memset(ones_r[:], 1.0)

            # big loads split in halves across both HWDGE rings, each half its
            # own tile so downstream compute starts as soon as its half lands
            HS = N_SPOTS // 2
            embT_a = constp.tile([P, HS], F32)
            embT_b = constp.tile([P, HS], F32)
            nc.sync.dma_start(out=embT_a[:], in_=embT[:, 0:HS])
            nc.scalar.dma_start(out=embT_b[:], in_=embT[:, HS:])

            def embT_cols(lo, width):
                # view into the correct half-tile (never straddles: callers
                # use 512-aligned slices within one half)
                if lo < HS:
                    return embT_a[:, lo : lo + width]
                return embT_b[:, lo - HS : lo - HS + width]

            th_sb = constp.tile([P, N_SPOTS], F32)  # tanh(W h + b).T  [j, s]
            e_sb = constp.tile([P, KCH], F32)  # e in spot-partition layout
            es_sb = constp.tile([P, KCH], F32)  # rescaled e
            rowsum = constp.tile([P, 1], F32)
            tot_sb = constp.tile([1, 1], F32)
            sinv_sb = constp.tile([1, 1], F32)
            ss_sb = constp.tile([P, 1], F32)  # broadcast scale
            xhl = constp.tile([P, KCH * NX], F8)  # [Xhi es_hi | Xlo es_lo]/chunk

            # ---- prologue: logits ----
            # h.T [j, s] = (W.T).T @ emb.T ; tanh(+b) fused from PSUM
            for c in range(N_SPOTS // 512):
                ph = php.tile([P, 512], F32)
                nc.tensor.matmul(
                    out=ph[:], lhsT=wt_sb[:], rhs=embT_cols(c * 512, 512),
                    start=True, stop=True,
                )
                nc.scalar.activation(
                    out=th_sb[:, ts(c, 512)], in_=ph[:],
                    func=mybir.ActivationFunctionType.Tanh, bias=b_sb[:, 0:1],
                )
            # logits, transposed into spot-partition layout:
            # logitsT chunk [128 s, 1] = th_chunk[j, s].T @ v
            pe = pep.tile([P, KCH], F32)
            for k in range(KCH):
                nc.tensor.matmul(
                    out=pe[:, k : k + 1], lhsT=th_sb[:, ts(k, P)], rhs=v_sb[:],
                    start=True, stop=True,
                )
            # e = exp(logits); accum_out gives the per-partition row sum free
            nc.scalar.activation(
                out=e_sb[:], in_=pe[:],
                func=mybir.ActivationFunctionType.Exp, accum_out=rowsum[:],
            )
            # global scale s = ESCALE / sum(e): partition-sum via ones matmul,
            # then reciprocal, then broadcast back across partitions
            ptot = psp.tile([1, 1], F32)
            nc.tensor.matmul(
                out=ptot[:], lhsT=ones_c[:, 0:1], rhs=rowsum[:, 0:1],
                start=True, stop=True,
            )
            nc.vector.tensor_copy(out=tot_sb[:], in_=ptot[:])
            nc.vector.reciprocal(out=sinv_sb[:], in_=tot_sb[:])
            pbc = psp.tile([P, 1], F32)
            nc.tensor.matmul(
                out=pbc[:], lhsT=ones_r[0:1, :], rhs=sinv_sb[0:1, 0:1],
                start=True, stop=True,
            )
            # es64 = 64 * es: carrying the x64 up front lets the lo-residual
            # build below be a single scalar_tensor_tensor per group
            nc.vector.tensor_scalar_mul(out=ss_sb[:], in0=pbc[:], scalar1=ESCALE * LOS)
            nc.vector.tensor_scalar_mul(
                out=es_sb[:], in0=e_sb[:], scalar1=ss_sb[:, 0:1]
            )

            # ---- X = [es*emb | es] as fp8 hi + lo (batched over chunk groups) ----
            # emb in spot-partition layout; scalar HWDGE ring so it runs in
            # parallel with the embT load on the sync ring
            HC = KCH * D // 2
            embc_a = constp.tile([P, HC], F32)
            embc_b = constp.tile([P, HC], F32)
            nc.sync.dma_start(out=embc_a[:], in_=embcp[:, 0:HC])
            nc.scalar.dma_start(out=embc_b[:], in_=embcp[:, HC:])
            xf = xfp.tile([P, KCH * D], F32)
            xhl3 = xhl[:].rearrange("p (k n) -> p k n", n=NX)
            xf3 = xf[:].rearrange("p (k d) -> p k d", d=D)
            emb3a = embc_a[:].rearrange("p (k d) -> p k d", d=D)
            emb3b = embc_b[:].rearrange("p (k d) -> p k d", d=D)
            es3 = es_sb[:].rearrange("p k -> p k ()")
            NG = 4  # build X in chunk groups so the main loop starts early
            GS = KCH // NG
            for g in range(NG):
                ks = slice(g * GS, (g + 1) * GS)
                if g < NG // 2:
                    embsrc = emb3a[:, ks, :]
                else:
                    embsrc = emb3b[:, slice(g * GS - KCH // 2, (g + 1) * GS - KCH // 2), :]
                ebc = es3[:, ks, :].to_broadcast([P, GS, D])
                # xf = 64*X (es_sb carries the x64); hi = fp8(xf/64) = fp8(X)
                nc.vector.tensor_mul(out=xf3[:, ks, :], in0=embsrc, in1=ebc)
                hi3 = xhl3[:, ks, 0:D]
                nc.scalar.activation(
                    out=hi3, in_=xf3[:, ks, :],
                    func=mybir.ActivationFunctionType.Copy, scale=1.0 / LOS,
                )
                # stored lo = 64*hi - 64*X = -64*(X - hi); sign folded back in
                # the epilogue. x64 keeps the residual out of fp8 subnormals.
                nc.vector.scalar_tensor_tensor(
                    out=xhl3[:, ks, D + 1 : NX - 1], in0=hi3, scalar=LOS,
                    in1=xf3[:, ks, :], op0=ALU.mult, op1=ALU.subtract,
                )
                nc.vector.tensor_scalar_mul(
                    out=xhl3[:, ks, D : D + 1], in0=es3[:, ks, :], scalar1=1.0 / LOS
                )
                nc.vector.scalar_tensor_tensor(
                    out=xhl3[:, ks, NX - 1 : NX], in0=xhl3[:, ks, D : D + 1],
                    scalar=LOS, in1=es3[:, ks, :], op0=ALU.mult, op1=ALU.subtract,
                )

            # ---- main loop: per gene tile, one N=258 DoubleRow matmul per
            # chunk-pair (hi and lo accumulate in separate PSUM columns).
            # Mask tiles are all resident (bufs=T) so every DMA can be queued
            # up front; queues are balanced to measured rates (gpsimd SW-DGE
            # ~160GB/s from t=0; SP/ACT HW-DGE ~200GB/s each, busy with emb
            # for the first ~10us; SP also carries the late out[t] DMAs).
            q_gp = {0, 1, 2, 4, 6, 9, 12, 15, 18}
            q_act = {3, 5, 7, 10, 13, 16, 19}
            for t in range(T):
                mt = maskp.tile([P, KCH * P], F8, name=f"mt{t}", tag="mt")
                eng = nc.gpsimd if t in q_gp else (nc.scalar if t in q_act else nc.sync)
                eng.dma_start(out=mt[:], in_=maskbt[t])
                mt3 = mt[:].rearrange("p (k g) -> p k g", g=P)
                pt = ptp.tile([P, NX], F32, name=f"pt{t}", tag="pt")
                for kk in range(0, KCH, 2):
                    k2 = slice(kk, kk + 2)
                    nc.tensor.matmul(
                        out=pt[:], lhsT=mt3[:, k2, :], rhs=xhl3[:, k2, 0:NX],
                        start=(kk == 0), stop=(kk == KCH - 2), perf_mode=DR,
                    )
                # s = hi - lo/64  (lo columns hold -64*residual); two ops since
                # only one DVE input may come from PSUM
                lo_sb = outp.tile([P, D + 1], F32, tag="lo_sb")
                nc.vector.tensor_scalar_mul(
                    out=lo_sb[:], in0=pt[:, D + 1 : NX], scalar1=-1.0 / LOS
                )
                s_sb = outp.tile([P, D + 1], F32, tag="s_sb")
                nc.vector.tensor_add(out=s_sb[:], in0=pt[:, 0 : D + 1], in1=lo_sb[:])
                rmax = outp.tile([P, 1], F32, tag="rmax")
                nc.vector.tensor_scalar_max(out=rmax[:], in0=s_sb[:, D : D + 1], scalar1=1e-37)
                rinv = outp.tile([P, 1], F32, tag="rinv")
                nc.vector.reciprocal(out=rinv[:], in_=rmax[:])
                o = outp.tile([P, D], F32, tag="o")
                nc.vector.tensor_scalar_mul(out=o[:], in0=s_sb[:, 0:D], scalar1=rinv[:, 0:1])
                nc.sync.dma_start(out=out[t], in_=o[:])

    nc.compile()
    return nc


def prep_inputs(spot_emb, W, b, v, gene_ids, spot_ids, T):
    """Host marshaling: shared fp32 operands + per-core fp8 mask slabs."""
    emb = np.ascontiguousarray(np.asarray(spot_emb, dtype=np.float32))
    W = np.asarray(W, dtype=np.float32)
    b = np.asarray(b, dtype=np.float32)
    v = np.asarray(v, dtype=np.float32)
    gene_ids = np.asarray(gene_ids).astype(np.int64)
    spot_ids = np.asarray(spot_ids).astype(np.int64)

    shared = {
        "embcp": np.ascontiguousarray(
            emb.reshape(KCH, P, D).transpose(1, 0, 2).reshape(P, KCH * D)
        ),
        "embT": np.ascontiguousarray(emb.T),
        "wt": np.ascontiguousarray(W.T),
        "bb": np.ascontiguousarray(b.reshape(D, 1)),
        "vv": np.ascontiguousarray(v.reshape(D, 1)),
    }

    # Dense 0/1 occupancy mask (set semantics: duplicate edges collapse),
    # built directly in the per-core padded layout: core c's genes live at
    # rows [c*T*P, c*T*P + G_PER); rows above G_PER stay zero padding.
    # Stored as the raw fp8e4m3 byte pattern (1.0 -> 0x38) so the DMA is a
    # plain byte copy with no dtype cast.
    g_pad = T * P
    M = np.zeros((N_CORES * g_pad, N_SPOTS), dtype=bool)
    pad_rows = (gene_ids // G_PER) * g_pad + (gene_ids % G_PER)
    M[pad_rows, spot_ids] = True
    # [c, t*128+g, k*128+p] -> [c, t, p, k, g]
    Mbt = M.reshape(N_CORES, T, P, KCH, P).transpose(0, 1, 4, 3, 2)
    Mbt = (
        np.ascontiguousarray(Mbt)
        .astype(np.uint8)
        .__mul__(np.uint8(0x38))
        .view(ml_dtypes.float8_e4m3)
        .reshape(N_CORES, T, P, KCH * P)
    )
    return [{"maskbt": Mbt[c], **shared} for c in range(N_CORES)]


_NC_CACHE = {}


def run(spot_emb, W, b, v, gene_ids, spot_ids, trace=False, **hw_kwargs):
    T = (G_PER + P - 1) // P  # 20
    key = T
    if key not in _NC_CACHE:
        nc = build_nc(T)
        nc.m = get_hw_module(nc.m)
        _NC_CACHE[key] = nc
    nc = _NC_CACHE[key]
    in_maps = prep_inputs(spot_emb, W, b, v, gene_ids, spot_ids, T)
    res = run_bass_kernel_spmd(
        nc, in_maps, core_ids=list(range(N_CORES)), trace=trace, **hw_kwargs
    )
    outs = [
        np.asarray(res.results[c]["out"], dtype=np.float32).reshape(T * P, D)[:G_PER]
        for c in range(N_CORES)
    ]
    full = np.concatenate(outs, axis=0)
    return full, res


def kernel(spot_emb, W, b, v, gene_ids, spot_ids, n_genes):
    n_genes = int(n_genes)
    assert n_genes == N_GENES, f"kernel hardcodes n_genes={N_GENES}, got {n_genes}"
    full, _ = run(spot_emb, W, b, v, gene_ids, spot_ids, trace=False)
    return full
